# revision 2
# baseline (speedup 1.0000x reference)
"""Two-layer GAT (DGL GATConv style) on 8 Trainium2 NeuronCores via Bass/Tile.

v2 strategy (dst-partitioned graph parallel, SWDGE-call-minimized):
  - Nodes split into 8 contiguous dst ranges; each core owns the edges whose
    dst falls in its range. Host relabels dsts (degree-balanced slots) and
    lays out BOTH layers' tables in the same relabeled slot order, so one
    (osrc, dloc) index set serves both layers.
  - Table rows are 272B [h bf16 x128 | el f32 x4]; er never ships: each core
    keeps er for its own slots in SBUF (from the projection matmul) and
    reconstructs per-edge er with a PE-transpose of the one-hot scatter
    matrix (s0T) and a tiny matmul per tile. This halves the number of
    1.1us-a-piece SWDGE indirect-DMA calls, which dominate the runtime.
  - Layer-1 aggregation runs TRANSPOSED (lhsT/rhs swapped): the accumulator
    is [h-col, slot], so the layer-2 projection (which needs h1^T as lhsT)
    fuses into the per-block finalize with no DRAM round-trip and no
    transpose phase between the layers.
"""
import sys

sys.path.insert(0, "/opt/trn_rl_repo")

import math
from contextlib import ExitStack

import ml_dtypes
import numpy as np

import concourse.bass as bass
import concourse.mybir as mybir
import concourse.tile as tile
from concourse._compat import with_exitstack

NEG = 0.2
ROWL = 136  # bf16 elems per table row: h(128) + el f32(as 8)
F = 128
H = 4
OUT = 32

bf16 = mybir.dt.bfloat16
f32 = mybir.dt.float32
i32 = mybir.dt.int32


class Cfg:
    def __init__(self, n, e, ncores, sup=32):
        self.N = n
        self.E = e
        self.NC = ncores
        self.NPC = n // ncores               # owned dsts per core
        self.NB = math.ceil(self.NPC / 128)  # dst blocks per core
        self.ROWS = self.NB * 128            # padded slice rows per core
        self.TROWS = self.ROWS * ncores
        self.SUP = sup                       # tiles per elementwise supertile
        # overflow-block packing puts all pad slots in the last block, so the
        # sentinel (the row pad edges gather) is the very last slice row.
        self.SENT = self.ROWS - 1


def prep_inputs(cfg, src, dst):
    """Host-side per-core edge layout, relabeled slot order for both layers.

    Per core:
      osrc [128, T] i32 : global table row (relabeled) of edge's src
      dloc [128, T] bf16: dst slot within its 128-dst block
    Shared: blk_of[t], k_of[t], Tb[b] block structure (identical across cores).
    """
    import heapq

    src = np.asarray(src).astype(np.int64)
    dst = np.asarray(dst).astype(np.int64)
    NPC, NB, ROWS, NC = cfg.NPC, cfg.NB, cfg.ROWS, cfg.NC

    def pack_core(counts):
        """Slot assignment: blocks 0..NB-2 load-capped at 1024 (8 tiles),
        the last block absorbs the overflow (it alone gets a 9th tile)."""
        total = int(counts.sum())
        order = np.argsort(-counts, kind="stable")
        degs = counts[order].astype(np.int64)
        n = len(order)
        nover = NPC - (NB - 1) * 128        # dsts in the overflow block
        D97 = max(total - (NB - 1) * 1024, 0) + 64
        csum = np.concatenate([[0], np.cumsum(degs)])
        wsums = csum[nover:] - csum[:-nover]
        i_best = int(np.argmin(np.abs(wsums - D97)))
        if wsums[i_best] < D97 and i_best > 0:
            i_best -= 1
        ov = set(range(i_best, i_best + nover))
        slot_of = np.empty(NPC, np.int64)
        loads = np.zeros(NB, np.int64)
        cnt = np.zeros(NB, np.int64)
        for j in range(i_best, i_best + nover):
            d = order[j]
            slot_of[d] = (NB - 1) * 128 + cnt[NB - 1]
            cnt[NB - 1] += 1
            loads[NB - 1] += degs[j]
        heap = [(0, b) for b in range(NB - 1)]
        heapq.heapify(heap)
        for j in range(n):
            if j in ov:
                continue
            d = order[j]
            while True:
                load, b = heapq.heappop(heap)
                if load != loads[b]:
                    continue  # stale entry
                if cnt[b] < 128:
                    slot_of[d] = b * 128 + cnt[b]
                    cnt[b] += 1
                    loads[b] += degs[j]
                    if cnt[b] < 128:
                        heapq.heappush(heap, (loads[b], b))
                    break
        return slot_of

    per_core = []
    perms = []
    cnts = np.zeros((NC, NB), np.int64)
    for c in range(NC):
        eid = np.nonzero(dst // NPC == c)[0]
        d0 = (dst[eid] - c * NPC).astype(np.int64)
        counts = np.bincount(d0, minlength=NPC)
        perm = pack_core(counts)
        rd = perm[d0]
        eorder = np.argsort(rd, kind="stable")
        esrc, erd = src[eid][eorder], rd[eorder]
        bid = erd // 128
        cnts[c] = np.bincount(bid, minlength=NB)
        per_core.append((esrc, erd, bid))
        perms.append(perm)

    Tb = np.maximum(1, np.ceil(cnts.max(axis=0) / 128).astype(np.int64))
    T = int(Tb.sum())
    t0 = np.concatenate([[0], np.cumsum(Tb)])[:-1]
    blk_of = np.concatenate([np.full(Tb[b], b, np.int64) for b in range(NB)])
    k_of = np.concatenate([np.arange(Tb[b]) for b in range(NB)])

    permcat = np.concatenate(perms)  # [NC*NPC] slot of (core, localid)

    ins = []
    for c in range(NC):
        esrc, erd, bid = per_core[c]
        osrc = np.full((128, T), cfg.SENT, np.int32)  # core-0 sentinel row
        dl = np.full((128, T), 127.0, np.float32)
        boff = np.concatenate([[0], np.cumsum(np.bincount(bid, minlength=NB))])
        score = esrc // NPC
        srow = score * ROWS + permcat[esrc]  # relabeled src table row
        for b in range(NB):
            j = np.arange(boff[b], boff[b + 1]) - boff[b]
            cols = t0[b] + j // 128
            parts = j % 128
            sl = slice(boff[b], boff[b + 1])
            osrc[parts, cols] = srow[sl]
            dl[parts, cols] = (erd[sl] % 128).astype(np.float32)
        ins.append(dict(osrc=osrc, dloc=dl.astype(ml_dtypes.bfloat16)))
    return ins, perms, Tb.tolist(), T, blk_of.tolist(), k_of.tolist()


def aug_weights(W, al, ar):
    """[128, 136] f32: [W | W@al_h | W@ar_h]."""
    Wa = np.zeros((F, 136), np.float32)
    Wa[:, :F] = W
    for h in range(H):
        Wa[:, F + h] = W[:, h * OUT:(h + 1) * OUT] @ al[h]
        Wa[:, F + H + h] = W[:, h * OUT:(h + 1) * OUT] @ ar[h]
    return Wa


@with_exitstack
def build_kernel(ctx: ExitStack, tc: tile.TileContext, cfg, Tb, T, blk_of, k_of,
                 dbg=False):
    nc = tc.nc
    NB, ROWS, TROWS, SUP = cfg.NB, cfg.ROWS, cfg.TROWS, cfg.SUP
    SENT = cfg.SENT

    dbgt = {}
    if dbg:
        for nm, shape, dt in (
                ("d_g", [128, SUP * ROWL], bf16),
                ("d_s0", [128, SUP * 128], bf16),
                ("d_s0T", [128, 512], bf16),
                ("d_erp", [128, SUP * 4], f32),
                ("d_lrl", [128, SUP * 4], f32),
                ("d_p", [128, SUP * 4], bf16),
                ("d_rhs", [128, SUP * 132], bf16),
                ("d_accH", [128, 128], f32),
                ("d_accP", [4, 128], f32),
                ("d_invf", [128, 128], f32),
                ("d_h1T", [128, 128], f32),
                ("d_er1", [128, 392], bf16),
                ("d_er2", [128, 392], bf16),
                ("d_sl1", [ROWS, ROWL], bf16),
                ("d_sl2", [ROWS, ROWL], bf16)):
            dbgt[nm] = nc.dram_tensor(nm, shape, dt, kind="ExternalOutput")

    # --- I/O ---
    featT = nc.dram_tensor("featT", [F, ROWS], f32, kind="ExternalInput")
    w1 = nc.dram_tensor("w1aug", [F, 136], f32, kind="ExternalInput")
    w2 = nc.dram_tensor("w2aug", [F, 136], f32, kind="ExternalInput")
    osrc = nc.dram_tensor("osrc", [128, T], i32, kind="ExternalInput")
    dlocd = nc.dram_tensor("dloc", [128, T], bf16, kind="ExternalInput")
    sentcd = nc.dram_tensor("sentcol", [128, 1], f32, kind="ExternalInput")
    iotad = nc.dram_tensor("iotarep", [128, SUP * 128], bf16,
                           kind="ExternalInput")
    ehotd = nc.dram_tensor("ehot", [4, 128], f32, kind="ExternalInput")
    identd = nc.dram_tensor("identb", [128, 128], bf16, kind="ExternalInput")
    b1d = nc.dram_tensor("b1col", [128, 1], f32, kind="ExternalInput")
    b2d = nc.dram_tensor("b2rep", [128, OUT], f32, kind="ExternalInput")
    out_ext = nc.dram_tensor("out", [ROWS, OUT], f32, kind="ExternalOutput")

    slice1 = nc.dram_tensor("slice1", [ROWS, ROWL], bf16)
    slice2 = nc.dram_tensor("slice2", [ROWS, ROWL], bf16)
    table1 = nc.dram_tensor("table1", [TROWS, ROWL], bf16, addr_space="Shared")
    table2 = nc.dram_tensor("table2", [TROWS, ROWL], bf16, addr_space="Shared")

    core_ids = list(range(cfg.NC))

    consts = ctx.enter_context(tc.tile_pool(name="consts", bufs=1))
    featp = ctx.enter_context(tc.tile_pool(name="featp", bufs=1))
    offp = ctx.enter_context(tc.tile_pool(name="offp", bufs=1))
    projp = ctx.enter_context(tc.tile_pool(name="projp", bufs=2))
    projps = ctx.enter_context(tc.tile_pool(name="projps", bufs=1, space="PSUM"))
    gp = ctx.enter_context(tc.tile_pool(name="gp", bufs=2))
    ep = ctx.enter_context(tc.tile_pool(name="ep", bufs=2))
    stp = ctx.enter_context(tc.tile_pool(name="stp", bufs=2))
    trp = ctx.enter_context(tc.tile_pool(name="trp", bufs=1, space="PSUM"))
    erpp = ctx.enter_context(tc.tile_pool(name="erpp", bufs=1, space="PSUM"))
    # accH and accP are two concurrently-open accumulation groups: they MUST
    # live in different PSUM banks (same-bank interleaving corrupts the open
    # group on HW).
    accp = ctx.enter_context(tc.tile_pool(name="accp", bufs=2, space="PSUM"))
    accp2 = ctx.enter_context(tc.tile_pool(name="accp2", bufs=2, space="PSUM"))
    finp = ctx.enter_context(tc.tile_pool(name="finp", bufs=1, space="PSUM"))
    outp = ctx.enter_context(tc.tile_pool(name="outp", bufs=2))

    w1_sb = consts.tile([F, 136], f32)
    w2_sb = consts.tile([F, 136], f32)
    iota_sb = consts.tile([128, SUP * 128], bf16)
    ehot_sb = consts.tile([4, 128], f32)
    b1_sb = consts.tile([128, 1], f32)
    b2_sb = consts.tile([128, OUT], f32)
    identb = consts.tile([128, 128], bf16)
    sent_sb = consts.tile([128, 1], f32)
    er1_all = consts.tile([128, NB * 4], bf16)
    er2_all = consts.tile([128, NB * 4], bf16)
    nc.sync.dma_start(w1_sb[:], w1[:])
    nc.sync.dma_start(w2_sb[:], w2[:])
    nc.sync.dma_start(iota_sb[:], iotad[:])
    nc.sync.dma_start(ehot_sb[:], ehotd[:])
    nc.sync.dma_start(b1_sb[:], b1d[:])
    nc.sync.dma_start(b2_sb[:], b2d[:])
    nc.sync.dma_start(sent_sb[:], sentcd[:])
    nc.sync.dma_start(identb[:], identd[:])

    featT_sb = featp.tile([F, ROWS], f32)
    nc.sync.dma_start(featT_sb[:], featT[:])
    osrc_sb = offp.tile([128, T], i32)
    dloc_sb = offp.tile([128, T], bf16)
    nc.sync.dma_start(osrc_sb[:], osrc[:])
    nc.sync.dma_start(dloc_sb[:], dlocd[:])

    def pack_row(ph, dst_dram, b):
        """psum [128,136] f32 -> packed bf16 row tile [h|el] -> DRAM slice.

        The last block holds the sentinel pad slot: fold el += sentcol
        (-1e9 on the sentinel partition) into the pack so pad edges gather
        an el that zeroes their softmax weight — no patch DMA needed."""
        row_t = projp.tile([128, ROWL], bf16, tag="rowt")
        rv = row_t[:, :].bitcast(f32)  # [128, 68] f32 view
        nc.vector.tensor_copy(row_t[:, 0:F], ph[:, 0:F])
        if b == NB - 1:
            sc = sent_sb[:, :]
            sc_ap = bass.AP(tensor=sc.tensor, offset=sc.offset,
                            ap=[sc.ap[0], [0, 4]])
            nc.vector.tensor_tensor(out=rv[:, 64:68], in0=ph[:, F:F + 4],
                                    in1=sc_ap, op=mybir.AluOpType.add)
        else:
            nc.vector.tensor_copy(rv[:, 64:68], ph[:, F:F + 4])
        nc.sync.dma_start(dst_dram[b * 128:(b + 1) * 128, :], row_t[:])

    # ---------- Phase P1: project own slice (relabeled order) with W1_aug ----
    for nt in range(NB):
        ph = projps.tile([128, 136], f32, tag="ph")
        nc.tensor.matmul(out=ph[:], lhsT=featT_sb[:, nt * 128:(nt + 1) * 128],
                         rhs=w1_sb[:], start=True, stop=True)
        nc.vector.tensor_copy(er1_all[:, nt * 4:(nt + 1) * 4], ph[:, 132:136])
        pack_row(ph, slice1, nt)

    tc.strict_bb_all_engine_barrier()
    nc.gpsimd.collective_compute(
        "AllGather", mybir.AluOpType.bypass, replica_groups=[core_ids],
        ins=[slice1[:]], outs=[table1[:]])
    tc.strict_bb_all_engine_barrier()

    # ---------- Edge phase ----------
    def finalize1(accH, accP, b, dump=False):
        """accH [h-col, slot] + accP [4, slot] sums -> h1T -> P2 -> slice2."""
        if dump:
            accd = outp.tile([128, 128], f32, tag="daccH")
            nc.vector.tensor_copy(accd[:], accH[:])
            nc.sync.dma_start(dbgt["d_accH"][:], accd[:])
            accd2 = outp.tile([4, 128], f32, tag="daccP")
            nc.vector.tensor_copy(accd2[:], accP[0:4, :])
            nc.sync.dma_start(dbgt["d_accP"][:], accd2[:])
        seps = outp.tile([4, 128], f32, tag="seps")
        nc.vector.tensor_scalar_add(seps[:], accP[0:4, :], 1e-30)
        invs = outp.tile([4, 128], f32, tag="invs")
        nc.vector.reciprocal(invs[:], seps[:])
        invf_ps = finp.tile([128, 128], f32, tag="invf")
        nc.tensor.matmul(out=invf_ps[:], lhsT=ehot_sb[:], rhs=invs[:],
                         start=True, stop=True)
        invf = outp.tile([128, 128], f32, tag="invfs")
        nc.vector.tensor_copy(invf[:], invf_ps[:])
        h1T = outp.tile([128, 128], f32, tag="h1T")
        nc.vector.tensor_tensor(out=h1T[:], in0=accH[:], in1=invf[:],
                                op=mybir.AluOpType.mult)
        b1v = b1_sb[:, :]
        b1_ap = bass.AP(tensor=b1v.tensor, offset=b1v.offset,
                        ap=[b1v.ap[0], [0, 128]])
        nc.vector.tensor_tensor(out=h1T[:], in0=h1T[:], in1=b1_ap,
                                op=mybir.AluOpType.add)
        nc.vector.tensor_scalar_max(h1T[:], h1T[:], 0.0)
        if dump:
            nc.sync.dma_start(dbgt["d_invf"][:], invf[:])
            nc.sync.dma_start(dbgt["d_h1T"][:], h1T[:])
        ph2 = projps.tile([128, 136], f32, tag="ph")
        nc.tensor.matmul(out=ph2[:], lhsT=h1T[:], rhs=w2_sb[:],
                         start=True, stop=True)
        nc.vector.tensor_copy(er2_all[:, b * 4:(b + 1) * 4], ph2[:, 132:136])
        pack_row(ph2, slice2, b)

    def finalize2(acc, b):
        s_eps = outp.tile([128, 4], f32, tag="seps2")
        nc.vector.tensor_scalar_add(s_eps[:], acc[:, 128:132], 1e-30)
        inv = outp.tile([128, 4], f32, tag="inv2")
        nc.vector.reciprocal(inv[:], s_eps[:])
        nc.vector.tensor_scalar_mul(inv[:], inv[:], 0.25)
        iv = inv[:, :]
        iv_ap = bass.AP(tensor=iv.tensor, offset=iv.offset,
                        ap=[iv.ap[0], [1, 4], [0, OUT]])
        tmp = outp.tile([128, F], f32, tag="tmp2")
        nc.vector.tensor_tensor(out=tmp[:], in0=acc[:, :F], in1=iv_ap,
                                op=mybir.AluOpType.mult)
        om = outp.tile([128, OUT], f32, tag="om")
        tv = tmp[:, :]
        tv_ap = bass.AP(tensor=tv.tensor, offset=tv.offset,
                        ap=[tv.ap[0], [1, OUT], [OUT, 4]])
        nc.vector.tensor_reduce(out=om[:], in_=tv_ap,
                                axis=mybir.AxisListType.X,
                                op=mybir.AluOpType.add)
        nc.vector.tensor_tensor(out=om[:], in0=om[:], in1=b2_sb[:],
                                op=mybir.AluOpType.add)
        nc.sync.dma_start(out_ext[b * 128:(b + 1) * 128, :], om[:])

    def edge_phase(table, layer, er_all):
        acc_box = [None]
        for t0 in range(0, T, SUP):
            K = min(SUP, T - t0)
            dbg0 = dbg and layer == 1 and t0 == 0
            # one SWDGE call per 128-edge tile (HW reads 1 offset/partition)
            g = gp.tile([128, SUP * ROWL], bf16, tag="g")
            for k in range(K):
                t = t0 + k
                nc.gpsimd.indirect_dma_start(
                    out=g[:, k * ROWL:(k + 1) * ROWL], out_offset=None,
                    in_=table[:],
                    in_offset=bass.IndirectOffsetOnAxis(
                        ap=osrc_sb[:, t:t + 1], axis=0))

            # one-hot scatter matrix for the whole supertile
            s0 = ep.tile([128, SUP * 128], bf16, tag="s0")
            dl = dloc_sb[:, t0:t0 + K]
            dloc_ap = bass.AP(tensor=dl.tensor, offset=dl.offset,
                              ap=[dl.ap[0], [1, K], [0, 128]])
            nc.vector.tensor_tensor(out=s0[:, :K * 128],
                                    in0=iota_sb[:, :K * 128], in1=dloc_ap,
                                    op=mybir.AluOpType.is_equal)

            # per-edge er via s0T (PE transpose, batched 4/psum-bank) + matmul
            erp = erpp.tile([128, SUP * 4], f32, tag="erp")
            for q in range(0, K, 4):
                KB = min(4, K - q)
                tp = trp.tile([128, 512], bf16, tag="tp")
                for j in range(KB):
                    k = q + j
                    nc.tensor.transpose(out=tp[:, j * 128:(j + 1) * 128],
                                        in_=s0[:, k * 128:(k + 1) * 128],
                                        identity=identb[:])
                s0T = stp.tile([128, 512], bf16, tag="s0T")
                nc.vector.tensor_copy(s0T[:, :KB * 128], tp[:, :KB * 128])
                if dbg0 and q == 0:
                    nc.sync.dma_start(dbgt["d_s0T"][:], s0T[:])
                for j in range(KB):
                    k = q + j
                    b = blk_of[t0 + k]
                    nc.tensor.matmul(out=erp[:, k * 4:(k + 1) * 4],
                                     lhsT=s0T[:, j * 128:(j + 1) * 128],
                                     rhs=er_all[:, b * 4:(b + 1) * 4],
                                     start=True, stop=True)

            g32 = g[:, :].bitcast(f32)    # [128, SUP*68]
            logit = ep.tile([128, SUP * 4], f32, tag="logit")
            el_ap = bass.AP(tensor=g32.tensor, offset=g32.offset + 64,
                            ap=[g32.ap[0], [68, K], [1, 4]])
            nc.vector.tensor_tensor(out=logit[:, :K * 4], in0=el_ap,
                                    in1=erp[:, :K * 4],
                                    op=mybir.AluOpType.add)
            lrl = ep.tile([128, SUP * 4], f32, tag="lrl")
            nc.vector.tensor_scalar_mul(lrl[:, :K * 4], logit[:, :K * 4], NEG)
            nc.vector.tensor_tensor(out=lrl[:, :K * 4], in0=logit[:, :K * 4],
                                    in1=lrl[:, :K * 4], op=mybir.AluOpType.max)
            # clamp: sentinel logits are ~-2e8, outside the HW exp table range
            nc.vector.tensor_scalar_max(lrl[:, :K * 4], lrl[:, :K * 4], -80.0)
            p_t = ep.tile([128, SUP * 4], bf16, tag="p")
            nc.scalar.activation(p_t[:, :K * 4], lrl[:, :K * 4],
                                 mybir.ActivationFunctionType.Exp)
            if dbg0:
                nc.sync.dma_start(dbgt["d_g"][:], g[:])
                nc.sync.dma_start(dbgt["d_s0"][:], s0[:])
                erp_d = outp.tile([128, SUP * 4], f32, tag="derp")
                nc.vector.tensor_copy(erp_d[:], erp[:])
                nc.sync.dma_start(dbgt["d_erp"][:], erp_d[:])
                nc.sync.dma_start(dbgt["d_lrl"][:], lrl[:])
                nc.sync.dma_start(dbgt["d_p"][:], p_t[:])

            rhs = ep.tile([128, SUP * 132], bf16, tag="rhs")
            gb, pb, rb = g[:, :], p_t[:, :], rhs[:, :]
            for hh in range(H):
                in0 = bass.AP(tensor=gb.tensor, offset=gb.offset + hh * OUT,
                              ap=[gb.ap[0], [ROWL, K], [1, OUT]])
                in1 = bass.AP(tensor=pb.tensor, offset=pb.offset + hh,
                              ap=[pb.ap[0], [4, K], [0, OUT]])
                o = bass.AP(tensor=rb.tensor, offset=rb.offset + hh * OUT,
                            ap=[rb.ap[0], [132, K], [1, OUT]])
                nc.vector.tensor_tensor(out=o, in0=in0, in1=in1,
                                        op=mybir.AluOpType.mult)
            pco = bass.AP(tensor=rb.tensor, offset=rb.offset + 128,
                          ap=[rb.ap[0], [132, K], [1, 4]])
            pci = bass.AP(tensor=pb.tensor, offset=pb.offset,
                          ap=[pb.ap[0], [4, K], [1, 4]])
            nc.vector.tensor_copy(out=pco, in_=pci)
            if dbg0:
                nc.sync.dma_start(dbgt["d_rhs"][:], rhs[:])

            for k in range(K):
                t = t0 + k
                b = blk_of[t]
                st = k_of[t] == 0
                sp = k_of[t] == Tb[b] - 1
                if layer == 1:
                    if st:
                        acc_box[0] = (
                            accp.tile([128, 128], f32, tag="acc", name="acc"),
                            accp2.tile([128, 128], f32, tag="accP",
                                       name="accP"))
                    accH, accP = acc_box[0]
                    nc.tensor.matmul(
                        out=accH[:],
                        lhsT=rhs[:, k * 132:k * 132 + 128],
                        rhs=s0[:, k * 128:(k + 1) * 128], start=st, stop=sp)
                    nc.tensor.matmul(
                        out=accP[0:4, :],
                        lhsT=rhs[:, k * 132 + 128:k * 132 + 132],
                        rhs=s0[:, k * 128:(k + 1) * 128], start=st, stop=sp)
                    if sp:
                        finalize1(accH, accP, b, dump=(dbg and b == 0))
                else:
                    if st:
                        acc_box[0] = accp.tile([128, 132], f32, tag="acc",
                                               name="acc")
                    acc = acc_box[0]
                    nc.tensor.matmul(
                        out=acc[:], lhsT=s0[:, k * 128:(k + 1) * 128],
                        rhs=rhs[:, k * 132:(k + 1) * 132], start=st, stop=sp)
                    if sp:
                        finalize2(acc, b)

    edge_phase(table1, 1, er1_all)
    tc.strict_bb_all_engine_barrier()
    nc.gpsimd.collective_compute(
        "AllGather", mybir.AluOpType.bypass, replica_groups=[core_ids],
        ins=[slice2[:]], outs=[table2[:]])
    tc.strict_bb_all_engine_barrier()
    edge_phase(table2, 2, er2_all)

    if dbg:
        tc.strict_bb_all_engine_barrier()
        nc.sync.dma_start(dbgt["d_er1"][:], er1_all[:])
        nc.sync.dma_start(dbgt["d_er2"][:], er2_all[:])
        for nt in range(NB):
            for srcd, dstd in ((slice1, dbgt["d_sl1"]), (slice2, dbgt["d_sl2"])):
                bt = projp.tile([128, ROWL], bf16, tag="dbgb", name="dbgb")
                nc.sync.dma_start(bt[:], srcd[nt * 128:(nt + 1) * 128, :])
                nc.sync.dma_start(dstd[nt * 128:(nt + 1) * 128, :], bt[:])


def build_nc(cfg, Tb, T, blk_of, k_of, compile=True, dbg=False):
    from concourse import bacc

    nc = bacc.Bacc("TRN2", target_bir_lowering=False)
    with tile.TileContext(nc) as tc:
        build_kernel(tc, cfg, Tb, T, blk_of, k_of, dbg=dbg)
    if compile:
        nc.compile()
    return nc


def make_in_maps(cfg, per_core_edges, perms, feat,
                 W1, al1, ar1, b1, W2, al2, ar2, b2):
    w1a = aug_weights(np.asarray(W1, np.float32), np.asarray(al1, np.float32),
                      np.asarray(ar1, np.float32))
    w2a = aug_weights(np.asarray(W2, np.float32), np.asarray(al2, np.float32),
                      np.asarray(ar2, np.float32))
    iota = np.broadcast_to(np.arange(cfg.SUP * 128) % 128, (128, cfg.SUP * 128))
    iota = np.ascontiguousarray(iota.astype(ml_dtypes.bfloat16))
    ehot = np.zeros((4, 128), np.float32)
    for h in range(H):
        ehot[h, h * OUT:(h + 1) * OUT] = 1.0
    identm = np.eye(128, dtype=np.float32).astype(ml_dtypes.bfloat16)
    sentc = np.zeros((128, 1), np.float32)
    sentc[127, 0] = -1e9
    b1c = np.ascontiguousarray(np.asarray(b1, np.float32).reshape(128, 1))
    b2m = np.asarray(b2, np.float32).reshape(H, OUT).mean(axis=0)
    b2r = np.ascontiguousarray(np.broadcast_to(b2m.reshape(1, OUT), (128, OUT)))
    feat = np.asarray(feat, np.float32)
    in_maps = []
    for c in range(cfg.NC):
        fslice = np.zeros((F, cfg.ROWS), np.float32)
        fslice[:, perms[c]] = feat[c * cfg.NPC:(c + 1) * cfg.NPC].T
        m = dict(
            featT=fslice,
            w1aug=w1a, w2aug=w2a,
            osrc=per_core_edges[c]["osrc"],
            dloc=per_core_edges[c]["dloc"],
            iotarep=iota, ehot=ehot, identb=identm, b1col=b1c, b2rep=b2r,
            sentcol=sentc,
        )
        in_maps.append(m)
    return in_maps


_CACHE = {}


def _get_program(cfg, src, dst, dbg=False):
    per_core, perms, Tb, T, blk_of, k_of = prep_inputs(cfg, src, dst)
    key = (cfg.N, cfg.E, cfg.NC, tuple(Tb), tuple(blk_of), tuple(k_of), dbg)
    if key not in _CACHE:
        _CACHE[key] = build_nc(cfg, Tb, T, blk_of, k_of, dbg=dbg)
    return _CACHE[key], per_core, perms


def kernel(feat, src, dst, W1, al1, ar1, b1, W2, al2, ar2, b2,
           _trace=False, _return_results=False, _dbg=False):
    from concourse.bass_utils import run_bass_kernel_spmd

    cfg = Cfg(100000, 800000, 8)
    nc, per_core, perms = _get_program(cfg, src, dst, dbg=_dbg)
    in_maps = make_in_maps(cfg, per_core, perms, feat, W1, al1, ar1, b1,
                           W2, al2, ar2, b2)
    res = run_bass_kernel_spmd(nc, in_maps, list(range(cfg.NC)), trace=_trace)
    out = np.zeros((cfg.N, OUT), np.float32)
    for c in range(cfg.NC):
        oc = np.asarray(res.results[c]["out"])  # [ROWS, 32], rows are slots
        out[c * cfg.NPC:(c + 1) * cfg.NPC] = oc[perms[c]]
    if _return_results:
        return out, res
    return out
